# revision 5
# baseline (speedup 1.0000x reference)
"""Trainium2 Bass kernel for nn_MemoryModule_541165879332 (scatter_memory).

Strategy: data-parallel over the token dim T=65536 across 8 NeuronCores
(8192 tokens each). The [M=256, C=256] memory and U/W weights are
replicated. Update path needs softmax over T -> two small AllReduces:
  AR1: sumexp over T               [128, 2]     (per memory slot)
  AR2: [add_mem partial | rawsum]  [128, 2, 257]
Read path is fully token-local.

Hardshrink is computed as raw = (e > lam*S) * e with the L1 norm folded
into a post-matmul row scale (mathematically identical to the reference
up to ~1e-7: the (x-l)/(x-l+1e-12) ratio is 1 to within float rounding
for any representable x > l).

Layouts (per core):
  qN  [128, 64, 256]  q blocks, token on partition    (reused as e_r in phase 2)
  qT  [128, 2, 8192]  q transposed, channel on partition
Phase 1 runs scores in [m, t] layout (memT stationary on PE, qT streams),
recomputing scores+exp after AR1 instead of keeping an 8MB e buffer.
Phase 2 runs scores in [t, m] layout (softmax over free dim).
"""

import os
import numpy as np

FP32 = np.float32

N_CORES = 8
B_FULL, S_FULL, C = 8, 8192, 256   # query [8, 8192, 256]
T_FULL = B_FULL * S_FULL           # 65536
TR = T_FULL // N_CORES             # 8192 tokens per core
M = 256
NB = TR // 128                     # 64 token blocks per core
LAM = 0.0025
EPS = 1e-12

_CACHE = {}


def _build():
    import concourse.bacc as bacc
    import concourse.mybir as mybir
    import concourse.tile as tile
    from concourse.masks import make_identity

    FP = mybir.dt.float32
    ALU = mybir.AluOpType
    AF = mybir.ActivationFunctionType

    nc = bacc.Bacc("TRN2", target_bir_lowering=False, debug=False,
                   num_devices=N_CORES)

    q_d = nc.dram_tensor("q", [TR, C], FP, kind="ExternalInput")
    mem_d = nc.dram_tensor("mem", [M, C], FP, kind="ExternalInput")
    uw_d = nc.dram_tensor("Uw", [C, C], FP, kind="ExternalInput")
    ub_d = nc.dram_tensor("Ub", [1, C], FP, kind="ExternalInput")
    ww_d = nc.dram_tensor("Ww", [C, C], FP, kind="ExternalInput")
    wb_d = nc.dram_tensor("Wb", [1, C], FP, kind="ExternalInput")
    rq_d = nc.dram_tensor("rq", [TR, 2 * C], FP, kind="ExternalOutput")
    attn_d = nc.dram_tensor("attn", [TR, M], FP, kind="ExternalOutput")
    nm_d = nc.dram_tensor("nm", [M, C], FP, kind="ExternalOutput")
    cc1_i = nc.dram_tensor("cc1_i", [128, 2], FP, kind="Internal")
    cc1_o = nc.dram_tensor("cc1_o", [128, 2], FP, kind="Internal", addr_space="Shared")
    cc2_i = nc.dram_tensor("cc2_i", [128, 2, C + 1], FP, kind="Internal")
    cc2_o = nc.dram_tensor("cc2_o", [128, 2, C + 1], FP, kind="Internal", addr_space="Shared")

    qv = q_d[:, :].rearrange("(n p) c -> p n c", p=128)         # [128, 64, 256]
    rqv = rq_d[:, :].rearrange("(n p) d -> p n d", p=128)       # [128, 64, 512]
    attnv = attn_d[:, :].rearrange("(n p) m -> p n m", p=128)   # [128, 64, 256]
    memv = mem_d[:, :].rearrange("(a p) c -> p a c", p=128)     # [128, 2, 256]
    uwv = uw_d[:, :].rearrange("(a p) c -> p a c", p=128)
    wwv = ww_d[:, :].rearrange("(a p) c -> p a c", p=128)
    nmv = nm_d[:, :].rearrange("(a p) c -> p a c", p=128)

    RG = [list(range(N_CORES))]
    dma = nc.sync.dma_start
    _pending_rqs = []

    with tile.TileContext(nc) as tc:
        with tc.tile_pool(name="big", bufs=1) as big, \
             tc.tile_pool(name="consts", bufs=1) as consts, \
             tc.tile_pool(name="scr", bufs=3) as scr, \
             tc.tile_pool(name="stats", bufs=1) as stats:

            qN = big.tile([128, NB, C], FP, tag="qN")     # 64 KB/part
            qT = big.tile([128, 2, TR], FP, tag="qT")     # 64 KB/part

            ident = consts.tile([128, 128], FP, tag="ident")
            memT = consts.tile([128, 2, M], FP, tag="memT")
            mem_sb = consts.tile([128, 2, C], FP, tag="mem_sb")
            make_identity(nc, ident[:, :])

            dma(out=qN[:, :, :], in_=qv)
            dma(out=mem_sb[:, :, :], in_=memv)
            # left half of read_query is just q
            dma(out=rqv[:, :, 0:C], in_=qN[:, :, :])

            # stats tiles
            Se1 = stats.tile([128, 2, 16], FP, tag="Se1")
            S1 = stats.tile([128, 2], FP, tag="S1")
            lamS = stats.tile([128, 2], FP, tag="lamS")
            rs1p = stats.tile([128, 2, 16], FP, tag="rs1p")
            cc2_sb = stats.tile([128, 2, C + 1], FP, tag="cc2_sb")
            P_sb = stats.tile([128, 2, C + 1], FP, tag="P_sb")

            with tc.tile_pool(name="ps_a", bufs=2, space="PSUM") as ps_a, \
                 tc.tile_pool(name="ps_t", bufs=2, space="PSUM") as ps_t, \
                 tc.tile_pool(name="ps_am", bufs=1, space="PSUM") as ps_am:

                # ---- memT = mem.T (4 PE transposes, 2 copies) ----
                for cc in range(2):
                    pm = ps_t.tile([128, M], FP, tag="pt1")
                    for mc in range(2):
                        nc.tensor.transpose(
                            pm[:, mc * 128:(mc + 1) * 128],
                            mem_sb[:, mc, cc * 128:(cc + 1) * 128],
                            ident[:, :])
                    nc.scalar.copy(memT[:, cc, :], pm[:, :])

                # ---- qT build: 128 PE transposes, 32 copies ----
                for g in range(32):   # 2 blocks per group
                    pt = ps_a.tile([128, 512], FP, tag="ps_a")
                    for cc in range(2):
                        for j in range(2):
                            blk = g * 2 + j
                            nc.tensor.transpose(
                                pt[:, (cc * 2 + j) * 128:(cc * 2 + j + 1) * 128],
                                qN[:, blk, cc * 128:(cc + 1) * 128],
                                ident[:, :])
                    nc.scalar.copy(
                        qT[:, :, g * 256:(g + 1) * 256],
                        pt[:, :].rearrange("p (a t) -> p a t", a=2))

                # ---- phase 1a: scores + exp, accumulate sumexp ----
                for mc in range(2):
                    for tz in range(16):
                        sp = ps_a.tile([128, 512], FP, tag="ps_a")
                        nc.tensor.matmul(sp, memT[:, 0, mc * 128:(mc + 1) * 128],
                                         qT[:, 0, tz * 512:(tz + 1) * 512],
                                         start=True, stop=False)
                        nc.tensor.matmul(sp, memT[:, 1, mc * 128:(mc + 1) * 128],
                                         qT[:, 1, tz * 512:(tz + 1) * 512],
                                         start=False, stop=True)
                        et = scr.tile([128, 512], FP, tag="e1")
                        nc.scalar.activation(et[:, :], sp[:, :], AF.Exp,
                                             accum_out=Se1[:, mc, tz:tz + 1])
                # total local sumexp -> AR1
                S1loc = stats.tile([128, 2], FP, tag="S1loc")
                nc.vector.reduce_sum(S1loc[:, :], Se1[:, :, :],
                                     axis=mybir.AxisListType.X)
                dma(out=cc1_i[:, :], in_=S1loc[:, :])
                nc.gpsimd.collective_compute(
                    "AllReduce", ALU.add, replica_groups=RG,
                    ins=[cc1_i[:, :]], outs=[cc1_o[:, :]])
                dma(out=S1[:, :], in_=cc1_o[:, :])
                nc.vector.tensor_scalar_mul(lamS[:, :], S1[:, :], LAM)

                # ---- phase 1b: recompute scores/exp, shrink, transpose, add_mem ----
                am_ps = [ps_am.tile([128, C], FP, tag=f"am{mc}", name=f"am_ps{mc}") for mc in range(2)]
                for mc in range(2):
                    for tz in range(16):
                        sp = ps_a.tile([128, 512], FP, tag="ps_a")
                        nc.tensor.matmul(sp, memT[:, 0, mc * 128:(mc + 1) * 128],
                                         qT[:, 0, tz * 512:(tz + 1) * 512],
                                         start=True, stop=False)
                        nc.tensor.matmul(sp, memT[:, 1, mc * 128:(mc + 1) * 128],
                                         qT[:, 1, tz * 512:(tz + 1) * 512],
                                         start=False, stop=True)
                        eb = scr.tile([128, 512], FP, tag="e1")
                        nc.scalar.activation(eb[:, :], sp[:, :], AF.Exp)
                        # raw = (e > lam*S) * e ; accumulate rawsum
                        nc.vector.scalar_tensor_tensor(
                            out=eb[:, :], in0=eb[:, :], scalar=lamS[:, mc:mc + 1],
                            in1=eb[:, :], op0=ALU.is_gt, op1=ALU.mult,
                            accum_out=rs1p[:, mc, tz:tz + 1])
                        pt = ps_t.tile([128, 512], FP, tag="pt1")
                        for j in range(4):
                            nc.tensor.transpose(pt[:, j * 128:(j + 1) * 128],
                                                eb[:, j * 128:(j + 1) * 128],
                                                ident[:, :])
                        rT = scr.tile([128, 512], FP, tag="rT")
                        nc.scalar.copy(rT[:, :], pt[:, :])
                        for j in range(4):
                            blk = tz * 4 + j
                            nc.tensor.matmul(
                                am_ps[mc], rT[:, j * 128:(j + 1) * 128],
                                qN[:, blk, :],
                                start=(tz == 0 and j == 0),
                                stop=(tz == 15 and j == 3))

                # pack [add_mem partial | rawsum] -> AR2
                rs1 = stats.tile([128, 2, 1], FP, tag="rs1")
                nc.vector.reduce_sum(rs1[:, :, :], rs1p[:, :, :],
                                     axis=mybir.AxisListType.X)
                for mc in range(2):
                    nc.scalar.copy(cc2_sb[:, mc, 0:C], am_ps[mc][:, :])
                nc.vector.tensor_copy(cc2_sb[:, :, C:C + 1], rs1[:, :, :])
                dma(out=cc2_i[:, :, :], in_=cc2_sb[:, :, :])
                nc.gpsimd.collective_compute(
                    "AllReduce", ALU.add, replica_groups=RG,
                    ins=[cc2_i[:, :, :]], outs=[cc2_o[:, :, :]])
                dma(out=P_sb[:, :, :], in_=cc2_o[:, :, :])

                # ---- phase 1c: finalize add_mem, gate, new_mem (replicated) ----
                rcpS = stats.tile([128, 2], FP, tag="rcpS")
                L1 = stats.tile([128, 2], FP, tag="L1")
                den = stats.tile([128, 2], FP, tag="den")
                rcden = stats.tile([128, 2], FP, tag="rcden")
                nc.vector.reciprocal(rcpS[:, :], S1[:, :])
                nc.vector.tensor_mul(L1[:, :], P_sb[:, :, C], rcpS[:, :])
                nc.vector.tensor_scalar_max(L1[:, :], L1[:, :], EPS)
                nc.vector.tensor_mul(den[:, :], S1[:, :], L1[:, :])
                nc.vector.reciprocal(rcden[:, :], den[:, :])
                am_sb = consts.tile([128, 2, C], FP, tag="am_sb")
                for mc in range(2):
                    nc.vector.tensor_scalar(
                        out=am_sb[:, mc, :], in0=P_sb[:, mc, 0:C],
                        scalar1=rcden[:, mc:mc + 1], scalar2=None, op0=ALU.mult)

                # U/W transposed; amT; gate
                UwT = consts.tile([128, 2, C], FP, tag="UwT")
                WwT = consts.tile([128, 2, C], FP, tag="WwT")
                amT = consts.tile([128, 2, C], FP, tag="amT")
                for (w_view, w_dst) in ((uwv, UwT), (wwv, WwT)):
                    w_sb = scr.tile([128, 2, C], FP, tag="wload")
                    dma(out=w_sb[:, :, :], in_=w_view)
                    for cc in range(2):
                        pw = ps_t.tile([128, M], FP, tag="pt1")
                        for fc in range(2):
                            nc.tensor.transpose(
                                pw[:, fc * 128:(fc + 1) * 128],
                                w_sb[:, fc, cc * 128:(cc + 1) * 128],
                                ident[:, :])
                        nc.scalar.copy(w_dst[:, cc, :], pw[:, :])
                for cc in range(2):
                    pa = ps_t.tile([128, M], FP, tag="pt1")
                    for mc in range(2):
                        nc.tensor.transpose(pa[:, mc * 128:(mc + 1) * 128],
                                            am_sb[:, mc, cc * 128:(cc + 1) * 128],
                                            ident[:, :])
                    nc.scalar.copy(amT[:, cc, :], pa[:, :])

                ub_sb = stats.tile([1, C], FP, tag="ub_sb")
                wb_sb = stats.tile([1, C], FP, tag="wb_sb")
                biasUW = stats.tile([1, C], FP, tag="biasUW")
                ones_row = stats.tile([1, 128], FP, tag="ones_row")
                dma(out=ub_sb[:, :], in_=ub_d[:, :])
                dma(out=wb_sb[:, :], in_=wb_d[:, :])
                nc.vector.tensor_add(biasUW[:, :], ub_sb[:, :], wb_sb[:, :])
                nc.vector.memset(ones_row[:, :], 1.0)

                g_sb = consts.tile([128, 2, C], FP, tag="g_sb")
                for mc in range(2):
                    gp = ps_am.tile([128, C], FP, tag=f"am{mc}")
                    nc.tensor.matmul(gp, memT[:, 0, mc * 128:(mc + 1) * 128],
                                     UwT[:, 0, :], start=True, stop=False)
                    nc.tensor.matmul(gp, memT[:, 1, mc * 128:(mc + 1) * 128],
                                     UwT[:, 1, :], start=False, stop=False)
                    nc.tensor.matmul(gp, amT[:, 0, mc * 128:(mc + 1) * 128],
                                     WwT[:, 0, :], start=False, stop=False)
                    nc.tensor.matmul(gp, amT[:, 1, mc * 128:(mc + 1) * 128],
                                     WwT[:, 1, :], start=False, stop=False)
                    nc.tensor.matmul(gp, ones_row[:, :], biasUW[:, :],
                                     start=False, stop=True)
                    # sigmoid(x) = 1/(1+exp(-x)) (avoids a second ACT table set)
                    nc.scalar.activation(g_sb[:, mc, :], gp[:, :], AF.Exp,
                                         scale=-1.0)
                nc.vector.tensor_scalar_add(g_sb[:, :, :], g_sb[:, :, :], 1.0)
                g2 = consts.tile([128, 2, C], FP, tag="g2")
                nc.vector.reciprocal(g2[:, :, :], g_sb[:, :, :])

                nm_sb = consts.tile([128, 2, C], FP, tag="nm_sb")
                dd = consts.tile([128, 2, C], FP, tag="dd")
                nc.vector.tensor_sub(dd[:, :, :], am_sb[:, :, :], mem_sb[:, :, :])
                nc.vector.tensor_mul(dd[:, :, :], g2[:, :, :], dd[:, :, :])
                nc.vector.tensor_add(nm_sb[:, :, :], mem_sb[:, :, :], dd[:, :, :])
                dma(out=nmv, in_=nm_sb[:, :, :])

                nmT = consts.tile([128, 2, M], FP, tag="nmT")
                for cc in range(2):
                    pn = ps_t.tile([128, M], FP, tag="pt1")
                    for mc in range(2):
                        nc.tensor.transpose(pn[:, mc * 128:(mc + 1) * 128],
                                            nm_sb[:, mc, cc * 128:(cc + 1) * 128],
                                            ident[:, :])
                    nc.scalar.copy(nmT[:, cc, :], pn[:, :])

            # ---- phase 2: read path, [t, m] layout, token-local ----
            Se2 = stats.tile([128, NB], FP, tag="Se2")
            rs2 = stats.tile([128, NB], FP, tag="rs2")
            sc2 = stats.tile([128, NB], FP, tag="sc2")
            lamS2 = stats.tile([128, NB], FP, tag="lamS2")
            ER = qN  # reuse the qN region for e_r / raw / attn

            BATCH = 16
            with tc.tile_pool(name="ps_r", bufs=2, space="PSUM") as ps_r, \
                 tc.tile_pool(name="ps_t2", bufs=2, space="PSUM") as ps_t2, \
                 tc.tile_pool(name="ps_am2", bufs=3, space="PSUM") as ps_am2, \
                 tc.tile_pool(name="aT", bufs=3) as aTp, \
                 tc.tile_pool(name="rqs", bufs=2) as rqsp:

                for bb in range(NB // BATCH):
                    b0 = bb * BATCH
                    # A: scores + exp (+sumexp)
                    for j in range(BATCH):
                        blk = b0 + j
                        sp = ps_r.tile([128, M], FP, tag="sr")
                        nc.tensor.matmul(sp, qT[:, 0, blk * 128:(blk + 1) * 128],
                                         nmT[:, 0, :], start=True, stop=False)
                        nc.tensor.matmul(sp, qT[:, 1, blk * 128:(blk + 1) * 128],
                                         nmT[:, 1, :], start=False, stop=True)
                        nc.scalar.activation(ER[:, blk, :], sp[:, :], AF.Exp,
                                             accum_out=Se2[:, blk:blk + 1])
                    # B: batch threshold
                    nc.vector.tensor_scalar_mul(lamS2[:, b0:b0 + BATCH],
                                                Se2[:, b0:b0 + BATCH], LAM)
                    # C: raw = (e > lam*Se)*e with rawsum accum
                    for j in range(BATCH):
                        blk = b0 + j
                        nc.vector.scalar_tensor_tensor(
                            out=ER[:, blk, :], in0=ER[:, blk, :],
                            scalar=lamS2[:, blk:blk + 1], in1=ER[:, blk, :],
                            op0=ALU.is_gt, op1=ALU.mult,
                            accum_out=rs2[:, blk:blk + 1])
                    # D: per-pair transposes + add_memory matmuls
                    for j2 in range(BATCH // 2):
                        t2 = ps_t2.tile([128, 512], FP, tag="pt2")
                        for jj in range(2):
                            blk = b0 + j2 * 2 + jj
                            for mc in range(2):
                                nc.tensor.transpose(
                                    t2[:, (jj * 2 + mc) * 128:(jj * 2 + mc + 1) * 128],
                                    ER[:, blk, mc * 128:(mc + 1) * 128],
                                    ident[:, :])
                        aT = aTp.tile([128, 512], FP, tag="aT")
                        nc.scalar.copy(aT[:, :], t2[:, :])
                        for jj in range(2):
                            blk = b0 + j2 * 2 + jj
                            am2 = ps_am2.tile([128, C], FP, tag="am2")
                            for mc in range(2):
                                nc.tensor.matmul(
                                    am2, aT[:, (jj * 2 + mc) * 128:(jj * 2 + mc + 1) * 128],
                                    nm_sb[:, mc, :],
                                    start=(mc == 0), stop=(mc == 1))
                            # unscaled drain (scale applied in SBUF once sc2 ready)
                            g4 = (blk % 4)
                            if g4 == 0:
                                rqs = rqsp.tile([128, 4, C], FP, tag="rqs")
                            nc.scalar.copy(rqs[:, g4, :], am2[:, :])
                            if g4 == 3:
                                _pending_rqs.append((blk - 3, rqs))
                    # E: batch scale: sc2 = 1/(Se*max(rawsum/Se, EPS))
                    t16a = scr.tile([128, BATCH], FP, tag="t16a")
                    t16b = scr.tile([128, BATCH], FP, tag="t16b")
                    nc.vector.reciprocal(t16a[:, :], Se2[:, b0:b0 + BATCH])
                    nc.vector.tensor_mul(t16b[:, :], rs2[:, b0:b0 + BATCH], t16a[:, :])
                    nc.vector.tensor_scalar_max(t16b[:, :], t16b[:, :], EPS)
                    nc.vector.tensor_mul(t16b[:, :], Se2[:, b0:b0 + BATCH], t16b[:, :])
                    nc.vector.reciprocal(sc2[:, b0:b0 + BATCH], t16b[:, :])
                    # F: scale rq halves and DMA out
                    for (blk0, rqs) in _pending_rqs:
                        for j in range(4):
                            blk = blk0 + j
                            nc.vector.tensor_scalar(
                                out=rqs[:, j, :], in0=rqs[:, j, :],
                                scalar1=sc2[:, blk:blk + 1], scalar2=None,
                                op0=ALU.mult)
                        dma(out=rqv[:, blk0:blk0 + 4, C:2 * C], in_=rqs[:, :, :])
                    _pending_rqs.clear()
                    # G: scale attn in place + DMA out
                    for j in range(BATCH):
                        blk = b0 + j
                        nc.vector.tensor_scalar(
                            out=ER[:, blk, :], in0=ER[:, blk, :],
                            scalar1=sc2[:, blk:blk + 1], scalar2=None, op0=ALU.mult)
                    dma(out=attnv[:, b0:b0 + BATCH, :], in_=ER[:, b0:b0 + BATCH, :])

    nc.compile()
    return nc


def _get_nc():
    if "nc" not in _CACHE:
        _CACHE["nc"] = _build()
    return _CACHE["nc"]


def kernel(query, mem, U_w, U_b, W_w, W_b):
    from concourse.bass_utils import run_bass_kernel_spmd

    nc = _get_nc()
    q = np.ascontiguousarray(np.asarray(query, dtype=FP32).reshape(T_FULL, C))
    mem = np.ascontiguousarray(np.asarray(mem, dtype=FP32))
    U_w = np.ascontiguousarray(np.asarray(U_w, dtype=FP32))
    W_w = np.ascontiguousarray(np.asarray(W_w, dtype=FP32))
    U_b = np.ascontiguousarray(np.asarray(U_b, dtype=FP32).reshape(1, C))
    W_b = np.ascontiguousarray(np.asarray(W_b, dtype=FP32).reshape(1, C))

    in_maps = []
    for r in range(N_CORES):
        in_maps.append({
            "q": np.ascontiguousarray(q[r * TR:(r + 1) * TR]),
            "mem": mem, "Uw": U_w, "Ub": U_b, "Ww": W_w, "Wb": W_b,
        })
    res = run_bass_kernel_spmd(nc, in_maps, core_ids=list(range(N_CORES)))
    rq = np.concatenate([res.results[r]["rq"] for r in range(N_CORES)], axis=0)
    attn = np.concatenate([res.results[r]["attn"] for r in range(N_CORES)], axis=0)
    new_mem = res.results[0]["nm"]
    read_query = rq.reshape(B_FULL, S_FULL, 2 * C)
    attn_r = attn.reshape(B_FULL, S_FULL, M)
    return read_query, attn_r, new_mem


# revision 8
# speedup vs baseline: 1.0222x; 1.0222x over previous
"""Trainium2 Bass kernel for nn_MemoryModule_541165879332 (scatter_memory).

Data-parallel over tokens T=65536 across 8 NeuronCores (8192 each); the
[256,256] memory and U/W weights are replicated. Softmax over T in the
update path -> two tiny collectives:
  CC1: AllGather of local sumexp [128,2] + local reduce
  CC2: AllReduce of [add_mem partial | rawsum] [128,2,257]
Read path is token-local.

Hardshrink: raw = (e > lam*S) * e, with the L1 normalization folded into
post-matmul row scales (identical to the reference to ~1e-7).

Per-core SBUF layout:
  qT  [128, 2, 8192] fp32  q channel-major (for score matmuls)
  E1  [128, 2, 8192] fp32  phase-1 e/raw in [m, t]; reused in phase 2 as
                           ER [128, 64, 256] e_r/raw/attn in [t, m]
  abf [128, 64, 256] bf16  phase-2 raw cast (xbar-transposed for the
                           bf16 add_memory matmul - an output-only path)
q itself is streamed from DRAM (not resident).
"""

import numpy as np

FP32 = np.float32

N_CORES = 8
B_FULL, S_FULL, C = 8, 8192, 256   # query [8, 8192, 256]
T_FULL = B_FULL * S_FULL           # 65536
TR = T_FULL // N_CORES             # 8192 tokens per core
M = 256
NB = TR // 128                     # 64 token blocks per core
LAM = 0.0025
EPS = 1e-12

_CACHE = {}


def _build():
    import concourse.bacc as bacc
    import concourse.mybir as mybir
    import concourse.tile as tile
    from concourse.masks import make_identity

    FP = mybir.dt.float32
    BF = mybir.dt.bfloat16
    ALU = mybir.AluOpType
    AF = mybir.ActivationFunctionType

    nc = bacc.Bacc("TRN2", target_bir_lowering=False, debug=False,
                   num_devices=N_CORES)

    q_d = nc.dram_tensor("q", [TR, C], FP, kind="ExternalInput")
    mem_d = nc.dram_tensor("mem", [M, C], FP, kind="ExternalInput")
    uw_d = nc.dram_tensor("Uw", [C, C], FP, kind="ExternalInput")
    ub_d = nc.dram_tensor("Ub", [1, C], FP, kind="ExternalInput")
    ww_d = nc.dram_tensor("Ww", [C, C], FP, kind="ExternalInput")
    wb_d = nc.dram_tensor("Wb", [1, C], FP, kind="ExternalInput")
    rq_d = nc.dram_tensor("rq", [TR, 2 * C], FP, kind="ExternalOutput")
    attn_d = nc.dram_tensor("attn", [TR, M], FP, kind="ExternalOutput")
    nm_d = nc.dram_tensor("nm", [M, C], FP, kind="ExternalOutput")
    cc1_i = nc.dram_tensor("cc1_i", [128, 2], FP, kind="Internal")
    cc1_o = nc.dram_tensor("cc1_o", [N_CORES * 128, 2], FP, kind="Internal",
                           addr_space="Shared")
    cc2_i = nc.dram_tensor("cc2_i", [128, 2, C + 1], FP, kind="Internal")
    cc2_o = nc.dram_tensor("cc2_o", [128, 2, C + 1], FP, kind="Internal",
                           addr_space="Shared")

    qv = q_d[:, :].rearrange("(n p) c -> p n c", p=128)         # [128, 64, 256]
    rqv = rq_d[:, :].rearrange("(n p) d -> p n d", p=128)       # [128, 64, 512]
    attnv = attn_d[:, :].rearrange("(n p) m -> p n m", p=128)   # [128, 64, 256]
    memv = mem_d[:, :].rearrange("(a p) c -> p a c", p=128)     # [128, 2, 256]
    uwv = uw_d[:, :].rearrange("(a p) c -> p a c", p=128)
    wwv = ww_d[:, :].rearrange("(a p) c -> p a c", p=128)
    nmv = nm_d[:, :].rearrange("(a p) c -> p a c", p=128)
    # AllGather output: rank r's [128, 2] at rows r*128..(r+1)*128
    cc1ov = cc1_o[:, :].rearrange("(r p) a -> p a r", p=128)    # [128, 2, 8]

    RG = [list(range(N_CORES))]
    dma = nc.sync.dma_start

    with tile.TileContext(nc) as tc:
        with tc.tile_pool(name="big", bufs=1) as big, \
             tc.tile_pool(name="consts", bufs=1) as consts, \
             tc.tile_pool(name="scr", bufs=5) as scr, \
             tc.tile_pool(name="ld", bufs=2) as ld, \
             tc.tile_pool(name="qb", bufs=3) as qbp, \
             tc.tile_pool(name="stats", bufs=1) as stats:

            qT = big.tile([128, 2, TR], FP, tag="qT")     # 64 KB/part
            E1 = big.tile([128, 2, TR], FP, tag="E1")     # 64 KB/part
            abf = big.tile([128, 16, C], BF, tag="abf")   # 8 KB/part (rotating)

            ident = consts.tile([128, 128], FP, tag="ident")
            memT = consts.tile([128, 2, M], FP, tag="memT")
            mem_sb = consts.tile([128, 2, C], FP, tag="mem_sb")
            make_identity(nc, ident[:, :])
            dma(out=mem_sb[:, :, :], in_=memv)

            Se1 = stats.tile([128, 2, 16], FP, tag="Se1")
            S1 = stats.tile([128, 2], FP, tag="S1")
            lamS = stats.tile([128, 2], FP, tag="lamS")
            rs1p = stats.tile([128, 2, 16], FP, tag="rs1p")
            cc2_sb = stats.tile([128, 2, C + 1], FP, tag="cc2_sb")
            P_sb = stats.tile([128, 2, C + 1], FP, tag="P_sb")

            with tc.tile_pool(name="ps_a", bufs=2, space="PSUM") as ps_a, \
                 tc.tile_pool(name="ps_t", bufs=2, space="PSUM") as ps_t, \
                 tc.tile_pool(name="ps_am", bufs=1, space="PSUM") as ps_am:

                # ---- memT = mem.T ----
                for cc in range(2):
                    pm = ps_t.tile([128, M], FP, tag="pt1")
                    for mc in range(2):
                        nc.tensor.transpose(
                            pm[:, mc * 128:(mc + 1) * 128],
                            mem_sb[:, mc, cc * 128:(cc + 1) * 128],
                            ident[:, :])
                    nc.scalar.copy(memT[:, cc, :], pm[:, :])

                # ---- load q in 8 chunks; emit rq left half; build qT ----
                for gq in range(16):
                    qc = ld.tile([128, 4, C], FP, tag="qc")
                    dma(out=qc[:, :, :], in_=qv[:, gq * 4:(gq + 1) * 4, :])
                    dma(out=rqv[:, gq * 4:(gq + 1) * 4, 0:C], in_=qc[:, :, :])
                    for g in range(2):       # 2 blocks per psum tile
                        pt = ps_a.tile([128, 512], FP, tag="ps_a")
                        for cc in range(2):
                            for j in range(2):
                                jj = g * 2 + j
                                nc.tensor.transpose(
                                    pt[:, (cc * 2 + j) * 128:(cc * 2 + j + 1) * 128],
                                    qc[:, jj, cc * 128:(cc + 1) * 128],
                                    ident[:, :])
                        t0 = gq * 512 + g * 256
                        nc.scalar.copy(
                            qT[:, :, t0:t0 + 256],
                            pt[:, :].rearrange("p (a t) -> p a t", a=2))

                # ---- phase 1a: scores -> exp (+sumexp) into E1 ----
                for mc in range(2):
                    for tz in range(16):
                        sp = ps_a.tile([128, 512], FP, tag="ps_a")
                        nc.tensor.matmul(sp, memT[:, 0, mc * 128:(mc + 1) * 128],
                                         qT[:, 0, tz * 512:(tz + 1) * 512],
                                         start=True, stop=False)
                        nc.tensor.matmul(sp, memT[:, 1, mc * 128:(mc + 1) * 128],
                                         qT[:, 1, tz * 512:(tz + 1) * 512],
                                         start=False, stop=True)
                        nc.scalar.activation(E1[:, mc, tz * 512:(tz + 1) * 512],
                                             sp[:, :], AF.Exp,
                                             accum_out=Se1[:, mc, tz:tz + 1])
                # local sumexp -> AllGather -> local total
                S1loc = stats.tile([128, 2], FP, tag="S1loc")
                nc.vector.reduce_sum(S1loc[:, :], Se1[:, :, :],
                                     axis=mybir.AxisListType.X)
                nc.gpsimd.dma_start(out=cc1_i[:, :], in_=S1loc[:, :])
                nc.gpsimd.collective_compute(
                    "AllGather", ALU.bypass, replica_groups=RG,
                    ins=[cc1_i[:, :]], outs=[cc1_o[:, :]])
                S1g = stats.tile([128, 2, N_CORES], FP, tag="S1g")
                nc.gpsimd.dma_start(out=S1g[:, :, :], in_=cc1ov)
                nc.vector.reduce_sum(S1[:, :], S1g[:, :, :],
                                     axis=mybir.AxisListType.X)
                nc.vector.tensor_scalar_mul(lamS[:, :], S1[:, :], LAM)

                # ---- phase 1b: shrink, transpose raw, add_mem partial ----
                am_ps = [ps_am.tile([128, C], FP, tag=f"am{mc}",
                                    name=f"am_ps{mc}") for mc in range(2)]
                for mc in range(2):
                    for tz in range(16):
                        sl = E1[:, mc, tz * 512:(tz + 1) * 512]
                        nc.vector.scalar_tensor_tensor(
                            out=sl, in0=sl, scalar=lamS[:, mc:mc + 1],
                            in1=sl, op0=ALU.is_gt, op1=ALU.mult,
                            accum_out=rs1p[:, mc, tz:tz + 1])
                        pt = ps_t.tile([128, 512], FP, tag="pt1")
                        for j in range(4):
                            nc.tensor.transpose(pt[:, j * 128:(j + 1) * 128],
                                                E1[:, mc, tz * 512 + j * 128:
                                                   tz * 512 + (j + 1) * 128],
                                                ident[:, :])
                        rT = scr.tile([128, 512], FP, tag="rT")
                        nc.scalar.copy(rT[:, :], pt[:, :])
                        for j in range(4):
                            blk = tz * 4 + j
                            qb = qbp.tile([128, C], FP, tag="qb")
                            nc.gpsimd.dma_start(out=qb[:, :],
                                                in_=qv[:, blk, :])
                            nc.tensor.matmul(
                                am_ps[mc], rT[:, j * 128:(j + 1) * 128],
                                qb[:, :],
                                start=(tz == 0 and j == 0),
                                stop=(tz == 15 and j == 3))

                # pack [add_mem partial | rawsum] -> AllReduce
                rs1 = stats.tile([128, 2, 1], FP, tag="rs1")
                nc.vector.reduce_sum(rs1[:, :, :], rs1p[:, :, :],
                                     axis=mybir.AxisListType.X)
                for mc in range(2):
                    nc.scalar.copy(cc2_sb[:, mc, 0:C], am_ps[mc][:, :])
                nc.vector.tensor_copy(cc2_sb[:, :, C:C + 1], rs1[:, :, :])
                nc.gpsimd.dma_start(out=cc2_i[:, :, :], in_=cc2_sb[:, :, :])
                nc.gpsimd.collective_compute(
                    "AllReduce", ALU.add, replica_groups=RG,
                    ins=[cc2_i[:, :, :]], outs=[cc2_o[:, :, :]])
                nc.gpsimd.dma_start(out=P_sb[:, :, :], in_=cc2_o[:, :, :])

                # ---- phase 1c: finalize add_mem, gate, new_mem ----
                rcpS = stats.tile([128, 2], FP, tag="rcpS")
                L1 = stats.tile([128, 2], FP, tag="L1")
                den = stats.tile([128, 2], FP, tag="den")
                rcden = stats.tile([128, 2], FP, tag="rcden")
                nc.vector.reciprocal(rcpS[:, :], S1[:, :])
                nc.vector.tensor_mul(L1[:, :], P_sb[:, :, C], rcpS[:, :])
                nc.vector.tensor_scalar_max(L1[:, :], L1[:, :], EPS)
                nc.vector.tensor_mul(den[:, :], S1[:, :], L1[:, :])
                nc.vector.reciprocal(rcden[:, :], den[:, :])
                am_sb = consts.tile([128, 2, C], FP, tag="am_sb")
                for mc in range(2):
                    nc.vector.tensor_scalar(
                        out=am_sb[:, mc, :], in0=P_sb[:, mc, 0:C],
                        scalar1=rcden[:, mc:mc + 1], scalar2=None, op0=ALU.mult)

                UwT = scr.tile([128, 2, C], FP, tag="rT", name="UwT")
                WwT = scr.tile([128, 2, C], FP, tag="rT", name="WwT")
                amT = consts.tile([128, 2, C], FP, tag="amT")
                for (w_view, w_dst) in ((uwv, UwT), (wwv, WwT)):
                    w_sb = scr.tile([128, 2, C], FP, tag="rT", name="w_sb")
                    dma(out=w_sb[:, :, :], in_=w_view)
                    for cc in range(2):
                        pw = ps_t.tile([128, M], FP, tag="pt1")
                        for fc in range(2):
                            nc.tensor.transpose(
                                pw[:, fc * 128:(fc + 1) * 128],
                                w_sb[:, fc, cc * 128:(cc + 1) * 128],
                                ident[:, :])
                        nc.scalar.copy(w_dst[:, cc, :], pw[:, :])
                for cc in range(2):
                    pa = ps_t.tile([128, M], FP, tag="pt1")
                    for mc in range(2):
                        nc.tensor.transpose(pa[:, mc * 128:(mc + 1) * 128],
                                            am_sb[:, mc, cc * 128:(cc + 1) * 128],
                                            ident[:, :])
                    nc.scalar.copy(amT[:, cc, :], pa[:, :])

                ub_sb = stats.tile([1, C], FP, tag="ub_sb")
                wb_sb = stats.tile([1, C], FP, tag="wb_sb")
                biasUW = stats.tile([1, C], FP, tag="biasUW")
                ones_row = stats.tile([1, 128], FP, tag="ones_row")
                dma(out=ub_sb[:, :], in_=ub_d[:, :])
                dma(out=wb_sb[:, :], in_=wb_d[:, :])
                nc.vector.tensor_add(biasUW[:, :], ub_sb[:, :], wb_sb[:, :])
                nc.vector.memset(ones_row[:, :], 1.0)

                g_sb = consts.tile([128, 2, C], FP, tag="g_sb")
                for mc in range(2):
                    gp = ps_am.tile([128, C], FP, tag=f"am{mc}", name=f"gp{mc}")
                    nc.tensor.matmul(gp, memT[:, 0, mc * 128:(mc + 1) * 128],
                                     UwT[:, 0, :], start=True, stop=False)
                    nc.tensor.matmul(gp, memT[:, 1, mc * 128:(mc + 1) * 128],
                                     UwT[:, 1, :], start=False, stop=False)
                    nc.tensor.matmul(gp, amT[:, 0, mc * 128:(mc + 1) * 128],
                                     WwT[:, 0, :], start=False, stop=False)
                    nc.tensor.matmul(gp, amT[:, 1, mc * 128:(mc + 1) * 128],
                                     WwT[:, 1, :], start=False, stop=False)
                    nc.tensor.matmul(gp, ones_row[:, :], biasUW[:, :],
                                     start=False, stop=True)
                    nc.scalar.activation(g_sb[:, mc, :], gp[:, :], AF.Exp,
                                         scale=-1.0)
                nc.vector.tensor_scalar_add(g_sb[:, :, :], g_sb[:, :, :], 1.0)
                g2 = scr.tile([128, 2, C], FP, tag="rT", name="g2")
                nc.vector.reciprocal(g2[:, :, :], g_sb[:, :, :])

                nm_sb = consts.tile([128, 2, C], FP, tag="nm_sb")
                dd = scr.tile([128, 2, C], FP, tag="rT", name="dd")
                nc.vector.tensor_sub(dd[:, :, :], am_sb[:, :, :], mem_sb[:, :, :])
                nc.vector.tensor_mul(dd[:, :, :], g2[:, :, :], dd[:, :, :])
                nc.vector.tensor_add(nm_sb[:, :, :], mem_sb[:, :, :], dd[:, :, :])
                dma(out=nmv, in_=nm_sb[:, :, :])

                nmT = consts.tile([128, 2, M], FP, tag="nmT")
                for cc in range(2):
                    pn = ps_t.tile([128, M], FP, tag="pt1")
                    for mc in range(2):
                        nc.tensor.transpose(pn[:, mc * 128:(mc + 1) * 128],
                                            nm_sb[:, mc, cc * 128:(cc + 1) * 128],
                                            ident[:, :])
                    nc.scalar.copy(nmT[:, cc, :], pn[:, :])
                nm_bf = consts.tile([128, 2, C], BF, tag="nm_bf")
                nc.vector.tensor_copy(nm_bf[:, :, :], nm_sb[:, :, :])

            # ---- phase 2: read path, [t, m] ----
            Se2 = stats.tile([128, NB], FP, tag="Se2")
            rs2 = stats.tile([128, NB], FP, tag="rs2")
            sc2 = stats.tile([128, NB], FP, tag="sc2")
            lamS2 = stats.tile([128, NB], FP, tag="lamS2")
            ER = E1[:, :, :].rearrange("p a t -> p (a t)").rearrange(
                "p (n m) -> p n m", m=C)                     # [128, 64, 256]

            BATCH = 16
            with tc.tile_pool(name="ps_r", bufs=3, space="PSUM") as ps_r, \
                 tc.tile_pool(name="ps_am2", bufs=4, space="PSUM") as ps_am2, \
                 tc.tile_pool(name="aT", bufs=3) as aTp, \
                 tc.tile_pool(name="rqs", bufs=2) as rqsp:

                for bb in range(NB // BATCH):
                    b0 = bb * BATCH
                    for j in range(BATCH):
                        blk = b0 + j
                        sp = ps_r.tile([128, M], FP, tag="sr")
                        nc.tensor.matmul(sp, qT[:, 0, blk * 128:(blk + 1) * 128],
                                         nmT[:, 0, :], start=True, stop=False)
                        nc.tensor.matmul(sp, qT[:, 1, blk * 128:(blk + 1) * 128],
                                         nmT[:, 1, :], start=False, stop=True)
                        nc.scalar.activation(ER[:, blk, :], sp[:, :], AF.Exp,
                                             accum_out=Se2[:, blk:blk + 1])
                    nc.vector.tensor_scalar_mul(lamS2[:, b0:b0 + BATCH],
                                                Se2[:, b0:b0 + BATCH], LAM)
                    for j in range(BATCH):
                        blk = b0 + j
                        nc.vector.scalar_tensor_tensor(
                            out=ER[:, blk, :], in0=ER[:, blk, :],
                            scalar=lamS2[:, blk:blk + 1], in1=ER[:, blk, :],
                            op0=ALU.is_gt, op1=ALU.mult,
                            accum_out=rs2[:, blk:blk + 1])
                    # cast raw->bf16 and xbar-transpose 4 blocks at a time
                    for h in range(BATCH // 4):
                        hb = b0 + h * 4
                        ab = (hb % 16)
                        nc.vector.tensor_copy(abf[:, ab:ab + 4, :],
                                              ER[:, hb:hb + 4, :])
                        aT8 = aTp.tile([128, 8, 128], BF, tag="aT8")
                        nc.sync.dma_start_transpose(
                            aT8[:, :, :],
                            abf[:, ab:ab + 4, :].rearrange("p n m -> p (n m)"))
                        for jj in range(4):
                            blk = hb + jj
                            am2 = ps_am2.tile([128, C], FP, tag="am2")
                            for mc in range(2):
                                nc.tensor.matmul(
                                    am2, aT8[:, jj * 2 + mc, :],
                                    nm_bf[:, mc, :],
                                    start=(mc == 0), stop=(mc == 1))
                            g4 = blk % 4
                            if g4 == 0:
                                rqs = rqsp.tile([128, 4, C], FP, tag="rqs")
                            nc.scalar.copy(rqs[:, g4, :], am2[:, :])
                            if g4 == 3:
                                _pending_rqs.append((blk - 3, rqs))
                    # batch scales: sc2 = 1/(Se*max(rawsum/Se, EPS))
                    t16a = scr.tile([128, BATCH], FP, tag="t16a")
                    t16b = scr.tile([128, BATCH], FP, tag="t16b")
                    nc.vector.reciprocal(t16a[:, :], Se2[:, b0:b0 + BATCH])
                    nc.vector.tensor_mul(t16b[:, :], rs2[:, b0:b0 + BATCH], t16a[:, :])
                    nc.vector.tensor_scalar_max(t16b[:, :], t16b[:, :], EPS)
                    nc.vector.tensor_mul(t16b[:, :], Se2[:, b0:b0 + BATCH], t16b[:, :])
                    nc.vector.reciprocal(sc2[:, b0:b0 + BATCH], t16b[:, :])
                    for (blk0, rqs) in _pending_rqs:
                        for j in range(4):
                            blk = blk0 + j
                            nc.vector.tensor_scalar(
                                out=rqs[:, j, :], in0=rqs[:, j, :],
                                scalar1=sc2[:, blk:blk + 1], scalar2=None,
                                op0=ALU.mult)
                        dma(out=rqv[:, blk0:blk0 + 4, C:2 * C], in_=rqs[:, :, :])
                    _pending_rqs.clear()
                    for j in range(BATCH):
                        blk = b0 + j
                        nc.vector.tensor_scalar(
                            out=ER[:, blk, :], in0=ER[:, blk, :],
                            scalar1=sc2[:, blk:blk + 1], scalar2=None, op0=ALU.mult)
                    dma(out=attnv[:, b0:b0 + BATCH, :], in_=ER[:, b0:b0 + BATCH, :])

    nc.compile()
    return nc


_pending_rqs = []


def _get_nc():
    if "nc" not in _CACHE:
        _CACHE["nc"] = _build()
    return _CACHE["nc"]


def kernel(query, mem, U_w, U_b, W_w, W_b):
    from concourse.bass_utils import run_bass_kernel_spmd

    nc = _get_nc()
    q = np.ascontiguousarray(np.asarray(query, dtype=FP32).reshape(T_FULL, C))
    mem = np.ascontiguousarray(np.asarray(mem, dtype=FP32))
    U_w = np.ascontiguousarray(np.asarray(U_w, dtype=FP32))
    W_w = np.ascontiguousarray(np.asarray(W_w, dtype=FP32))
    U_b = np.ascontiguousarray(np.asarray(U_b, dtype=FP32).reshape(1, C))
    W_b = np.ascontiguousarray(np.asarray(W_b, dtype=FP32).reshape(1, C))

    in_maps = []
    for r in range(N_CORES):
        in_maps.append({
            "q": np.ascontiguousarray(q[r * TR:(r + 1) * TR]),
            "mem": mem, "Uw": U_w, "Ub": U_b, "Ww": W_w, "Wb": W_b,
        })
    res = run_bass_kernel_spmd(nc, in_maps, core_ids=list(range(N_CORES)))
    rq = np.concatenate([res.results[r]["rq"] for r in range(N_CORES)], axis=0)
    attn = np.concatenate([res.results[r]["attn"] for r in range(N_CORES)], axis=0)
    new_mem = res.results[0]["nm"]
    read_query = rq.reshape(B_FULL, S_FULL, 2 * C)
    attn_r = attn.reshape(B_FULL, S_FULL, M)
    return read_query, attn_r, new_mem
